# revision 50
# baseline (speedup 1.0000x reference)
"""Causal attention (single head, S=4096, d=1024) on 8 TRN2 NeuronCores —
collective-free, transposed-score formulation.

Core i computes output rows {i + 8m} (strided sequence-parallel Q; perfectly
load-balanced). All cross-core communication is eliminated algebraically:
with W' := Wq^T Wk precomputed on the host (f32, cast bf16),

    S   = Q K^T = x W' x^T          (one device projection G' = x_q W')
    O   = A V   = (A x) Wv^T        (apply Wv once at the end)

Scores are computed directly TRANSPOSED (S^T[k, q] tiles, k on partitions):
    S^T tile = xt[:, di, kblk]^T-contraction with g[:, di, q-span]
so A^T is produced by exp() with no PE transposes, and feeds the AV matmul
(lhsT = A^T block) directly. Causal masking multiplies the diagonal-band
128-col region of each k-block tile by a 0/1 bf16 mask. Softmax row sums are
N=1 matmuls against a ones vector sharing the A^T weight loads of the AV
matmul; normalization scales AX rows (q on partitions) before the final
Wv projection. The final projection is split (q 0:384 early / 384:512 late)
to overlap with the last attention chunk; output is O^T in bf16.

Numerics: bf16 matmuls, f32 PSUM accumulation; exp skips max-subtraction
(scores/32 ~ N(0,1); masked entries are exactly 0 after the mask multiply).
"""

import numpy as np
import ml_dtypes

import concourse.bass as bass  # noqa: F401  (registers engines)
import concourse.mybir as mybir
from concourse import bacc, tile, masks
from concourse.bass_utils import run_bass_kernel_spmd

SEQ = 4096
D = 1024
N_CORES = 8
CORE_IDS = list(range(N_CORES))
QLOC = SEQ // N_CORES          # 512 q rows per core
NKB = SEQ // 128               # 32 k blocks of 128
OUT_SHAPE = (1024, 512)        # out dram tensor is O^T [D, QLOC] bf16
BF16 = mybir.dt.bfloat16
F32 = mybir.dt.float32
SM_SCALE = 1.0 / np.sqrt(np.float32(D))


def _emit_compute(nc, tc, pp, cp_tiles, io, rep):
    maskm_sb, ones_sb, warm_sb = cp_tiles
    xq, xt, xn, wpp, wvT, maskm, out = io

    with tc.tile_pool(name="persist", bufs=1) as pers:
        g_sb = pers.tile([128, 8, QLOC], BF16, name="g_sb")      # G'^T [din, q]
        axT_sb = pers.tile([128, 8, QLOC], BF16, name="axT_sb")  # (AX)^T
        ot_sb = pers.tile([128, 8, QLOC], BF16, name="ot_sb")    # O^T

        with (
            tc.tile_pool(name="xt", bufs=1) as xtp,
            tc.tile_pool(name="xn", bufs=1) as xnp,
        ):
            xt_sb = xtp.tile([128, 8, SEQ], BF16, name="xt_sb")   # X^T d-major
            xn_sb = xnp.tile([128, NKB, D], BF16, name="xn_sb")   # X seq-major
            wv_sb = xnp.tile([128, 8, D], BF16, name="wv_sb")
            xt_v = xt.rearrange("(a p) s -> p a s", p=128)
            xn_v = xn.rearrange("(blk p) d -> p blk d", p=128)
            wv_v = wvT.rearrange("(a p) n -> p a n", p=128)

            with tc.tile_pool(name="proj", bufs=1) as wp:
                xq_sb = wp.tile([128, 8, QLOC], BF16, name="xq_sb")
                wpp_sb = wp.tile([128, 8, D], BF16, name="wpp_sb")
                # PE p-state warmup: the tensor engine clock ramps to full
                # speed only after ~3us of continuous execution. Run dummy
                # matmuls (on scratch data, results unread) while the first
                # DMAs land so real work starts at 2.4 GHz.
                for w in range(24):
                    ps_w = pp.tile([128, 512], F32, tag="o", bufs=2,
                                   name=f"ps_warm{w}_{rep}")
                    nc.tensor.matmul(ps_w[:], warm_sb[:, 0:128], warm_sb[:],
                                     start=True, stop=True)
                # critical-path DMAs first: the tiny mask, then xq + wpp
                # (halved so the first G' group starts after ~1.5MB)
                xq_v = xq.rearrange("(a p) q -> p a q", p=128)
                wpp_v = wpp.rearrange("(a p) n -> p a n", p=128)
                nc.sync.dma_start(xq_sb[:, :, 0:256], xq_v[:, :, 0:256])
                nc.scalar.dma_start(wpp_sb[:, :, 0:512], wpp_v[:, :, 0:512])
                nc.sync.dma_start(xq_sb[:, :, 256:512], xq_v[:, :, 256:512])
                nc.scalar.dma_start(wpp_sb[:, :, 512:1024],
                                    wpp_v[:, :, 512:1024])
                nc.gpsimd.dma_start(maskm_sb[:], maskm[:])

                # bulk DMAs ordered by first use, on the idle SP/Pool queues
                def xt_dma(eng, c):
                    eng.dma_start(xt_sb[:, :, 512 * c:512 * (c + 1)],
                                  xt_v[:, :, 512 * c:512 * (c + 1)])

                def xn_dma(eng, g):
                    eng.dma_start(xn_sb[:, 4 * g:4 * (g + 1), :],
                                  xn_v[:, 4 * g:4 * (g + 1), :])

                xt_dma(nc.sync, 0); xt_dma(nc.gpsimd, 1)
                xt_dma(nc.sync, 2); xt_dma(nc.gpsimd, 3)
                xn_dma(nc.sync, 0); xn_dma(nc.gpsimd, 1)
                xt_dma(nc.sync, 4); xt_dma(nc.gpsimd, 5)
                xn_dma(nc.sync, 2); xn_dma(nc.gpsimd, 3)
                xt_dma(nc.sync, 6); xt_dma(nc.gpsimd, 7)
                xn_dma(nc.sync, 4); xn_dma(nc.gpsimd, 5)
                xn_dma(nc.sync, 6); xn_dma(nc.gpsimd, 7)
                # wv is consumed only by the last-emitted final projection
                nc.sync.dma_start(wv_sb[:, :, 0:512], wv_v[:, :, 0:512])
                nc.gpsimd.dma_start(wv_sb[:, :, 512:1024],
                                    wv_v[:, :, 512:1024])

                # --- G'^T = W'^T @ x_q^T : [1024 do', 512 q]
                for gi in range(8):
                    ps = pp.tile([128, 512], F32, tag="sacc", bufs=3,
                                 name=f"ps_g{gi}_{rep}")
                    for di in range(8):
                        nc.tensor.matmul(
                            ps[:], wpp_sb[:, di, 128 * gi:128 * (gi + 1)],
                            xq_sb[:, di, :], start=(di == 0), stop=(di == 7),
                        )
                    nc.scalar.copy(g_sb[:, gi, :], ps[:])

            with tc.tile_pool(name="attn", bufs=1) as ap:
                # A^T tiles, one per 128-wide k block; tile kb covers q-span
                # [128*(kb//8), 512): chunks kb//8..3 all need this k block.
                at = [
                    ap.tile([128, 512 - 128 * (kb // 8)], BF16,
                            name=f"at{kb}_{rep}")
                    for kb in range(NKB)
                ]
                rinv_all = ap.tile([128, 4], F32, name=f"rinv_all_{rep}")

                # band block j: columns qf < 16j of the band region are fully
                # masked; skip them in the matmul and zero them once so the
                # AV/sum reads see exact zeros.
                for kb in range(NKB):
                    j = kb % 8
                    if j > 0:
                        nc.vector.memset(at[kb][:, 0:16 * j], 0.0)

                def emit_scores(kb):
                    qlo = 128 * (kb // 8)
                    j = kb % 8
                    sk = 16 * j          # fully-masked leading band columns
                    nq = 512 - qlo - sk
                    ps_s = pp.tile([128, 512], F32, tag="sacc", bufs=3,
                                   name=f"ps_s{kb}_{rep}")
                    for di in range(8):
                        nc.tensor.matmul(
                            ps_s[:, 0:nq],
                            xt_sb[:, di, 128 * kb:128 * (kb + 1)],
                            g_sb[:, di, qlo + sk:512],
                            start=(di == 0), stop=(di == 7),
                        )
                    nc.scalar.activation(
                        at[kb][:, sk:512 - qlo], ps_s[:, 0:nq],
                        mybir.ActivationFunctionType.Exp, scale=float(SM_SCALE),
                    )
                    # partially-masked remainder of the diagonal band
                    nc.vector.tensor_mul(
                        at[kb][:, sk:128], at[kb][:, sk:128],
                        maskm_sb[:, j, sk:128],
                    )

                def emit_ax(b):
                    nkc = 8 * (b + 1)
                    ps_h0 = pp.tile([128, 512], F32, tag="ax0", bufs=1,
                                    name=f"ps_h0_{b}_{rep}")
                    ps_h1 = pp.tile([128, 512], F32, tag="ax1", bufs=1,
                                    name=f"ps_h1_{b}_{rep}")
                    ps_sum = pp.tile([128, 1], F32, tag="sum", bufs=1,
                                     name=f"ps_sum{b}_{rep}")
                    for kc in range(nkc):
                        qoff = 128 * b - 128 * (kc // 8)
                        lhs = at[kc][:, qoff:qoff + 128]
                        st, sp = (kc == 0), (kc == nkc - 1)
                        # sum first: it shares the loaded A^T weights with
                        # h0/h1, and the reciprocal overlaps the last two
                        nc.tensor.matmul(ps_sum[:], lhs, ones_sb[:],
                                         start=st, stop=sp)
                        nc.tensor.matmul(ps_h0[:], lhs, xn_sb[:, kc, 0:512],
                                         start=st, stop=sp)
                        nc.tensor.matmul(ps_h1[:], lhs, xn_sb[:, kc, 512:1024],
                                         start=st, stop=sp)
                    nc.vector.reciprocal(rinv_all[:, b:b + 1], ps_sum[:])
                    ax = ap.tile([128, D], BF16, tag="ax", bufs=4,
                                 name=f"ax{b}_{rep}")
                    # scale each half then immediately transpose it via the
                    # XBAR on the (otherwise idle after startup) scalar DMA
                    # queue; frees the PE and a PSUM bank
                    nc.vector.tensor_scalar_mul(ax[:, 0:512], ps_h0[:],
                                                rinv_all[:, b:b + 1])
                    nc.scalar.dma_start_transpose(
                        axT_sb[:, 0:4, 128 * b:128 * (b + 1)], ax[:, 0:512])
                    nc.vector.tensor_scalar_mul(ax[:, 512:1024], ps_h1[:],
                                                rinv_all[:, b:b + 1])
                    nc.scalar.dma_start_transpose(
                        axT_sb[:, 4:8, 128 * b:128 * (b + 1)],
                        ax[:, 512:1024])
                    return ax

                def emit_final(q0, q1, do):
                    # alternate PSUM tags: 4 rotating banks ("o" + the
                    # scores-stream banks, free by now) so short N=128
                    # groups aren't gated on the PSUM->SBUF copies
                    tag = "o" if do % 2 == 0 else "sacc"
                    ps = pp.tile([128, 512], F32, tag=tag,
                                 bufs=2 if tag == "o" else 3,
                                 name=f"ps_ot{do}_{q0}_{rep}")
                    for di in range(8):
                        nc.tensor.matmul(
                            ps[:, 0:q1 - q0],
                            wv_sb[:, di, 128 * do:128 * (do + 1)],
                            axT_sb[:, di, q0:q1],
                            start=(di == 0), stop=(di == 7),
                        )
                    if do % 2 == 0:
                        nc.scalar.copy(ot_sb[:, do, q0:q1], ps[:, 0:q1 - q0])
                    else:
                        nc.vector.tensor_copy(ot_sb[:, do, q0:q1],
                                              ps[:, 0:q1 - q0])

                # pipeline: scores stream (sums lagged one block) with
                # per-chunk AV / transposes interleaved
                for kb in range(8):
                    emit_scores(kb)
                emit_ax(0)
                for kb in range(8, 16):
                    emit_scores(kb)
                emit_ax(1)
                for kb in range(16, 24):
                    emit_scores(kb)
                emit_ax(2)
                for kb in range(24, 32):
                    emit_scores(kb)
                # final projection for q 0:384 overlaps the last chunk;
                # its output DMAs drain during AX(3)
                out_v = out.rearrange("(a p) q -> p a q", p=128)
                for do in range(8):
                    emit_final(0, 384, do)
                    eng = nc.sync if do % 2 == 0 else nc.gpsimd
                    eng.dma_start(out_v[:, do, 0:384], ot_sb[:, do, 0:384])
                emit_ax(3)
                for do in range(8):
                    emit_final(384, 512, do)
                    eng = nc.sync if do % 2 == 0 else nc.gpsimd
                    eng.dma_start(out_v[:, do, 384:512], ot_sb[:, do, 384:512])
                if _DBG_SINK[0] is not None:
                    nc.sync.dma_start(_DBG_SINK[0][:], rinv_all[:])


_DBG_SINK = [None]


def build_nc(reps=1, variant="full", loop=False, debug=False):
    nc = bacc.Bacc("TRN2", target_bir_lowering=False)
    _DBG_SINK[0] = (
        nc.dram_tensor("dbg", [128, 4], F32, kind="ExternalOutput")
        if debug else None
    )

    xq = nc.dram_tensor("xq", [D, QLOC], BF16, kind="ExternalInput")
    xt = nc.dram_tensor("xt", [D, SEQ], BF16, kind="ExternalInput")
    xn = nc.dram_tensor("xn", [SEQ, D], BF16, kind="ExternalInput")
    wpp = nc.dram_tensor("wpp", [D, D], BF16, kind="ExternalInput")
    wvT = nc.dram_tensor("wvT", [D, D], BF16, kind="ExternalInput")
    maskm = nc.dram_tensor("maskm", [128, 1024], BF16, kind="ExternalInput")
    out = nc.dram_tensor("out", [D, QLOC], BF16, kind="ExternalOutput")
    io = (xq, xt, xn, wpp, wvT, maskm, out)

    with tile.TileContext(nc) as tc:
        with (
            tc.tile_pool(name="const", bufs=1) as cp,
            tc.tile_pool(name="psum", bufs=2, space="PSUM") as pp,
        ):
            ones_sb = cp.tile([128, 1], BF16, name="ones_sb")
            nc.vector.memset(ones_sb[:], 1.0)
            maskm_sb = cp.tile([128, 8, 128], BF16, name="maskm_sb")
            # scratch operand for the p-state warmup matmuls (results unread)
            warm_sb = cp.tile([128, 512], BF16, name="warm_sb")
            nc.vector.memset(warm_sb[:], 0.0)
            cp_tiles = (maskm_sb, ones_sb, warm_sb)
            if loop:
                # hardware loop: body repeats `reps` times, all-engine
                # barrier between iterations (in For_i's reset block)
                with tc.For_i(0, reps):
                    _emit_compute(nc, tc, pp, cp_tiles, io, 0)
            else:
                for rep in range(reps):
                    if rep > 0:
                        # serialize reps so the R-slope measures single-shot
                        tc.strict_bb_all_engine_barrier()
                    _emit_compute(nc, tc, pp, cp_tiles, io, rep)

    nc.compile()
    return nc


def make_in_maps(x, Wq, Wk, Wv):
    x = np.asarray(x, dtype=np.float32)
    Wq = np.asarray(Wq, dtype=np.float32)
    Wk = np.asarray(Wk, dtype=np.float32)
    Wv = np.asarray(Wv, dtype=np.float32)

    bf = ml_dtypes.bfloat16
    xT = np.ascontiguousarray(x.T).astype(bf)          # [D, SEQ]
    xn = np.ascontiguousarray(x).astype(bf)            # [SEQ, D]
    wpp = np.ascontiguousarray(Wq.T @ Wk).astype(bf)   # W' = Wq^T Wk [din,din]
    wvT = np.ascontiguousarray(Wv.T).astype(bf)

    # 0/1 band mask, S^T orientation: for band block j (k rows 128j+p of the
    # chunk's 1024-span), q col qf (global q = 8*qf + i within the span):
    # valid iff 128j + p <= 8*qf + i.
    p = np.arange(128)[:, None, None]
    j = np.arange(8)[None, :, None]
    qf = np.arange(128)[None, None, :]
    in_maps = []
    for i in CORE_IDS:
        m = (128 * j + p <= 8 * qf + i).astype(bf).reshape(128, 1024)
        in_maps.append({
            "xq": np.ascontiguousarray(xT[:, i::N_CORES]),
            "xt": xT, "xn": xn,
            "wpp": wpp, "wvT": wvT,
            "maskm": np.ascontiguousarray(m),
        })
    return in_maps


def assemble(results):
    out = np.empty((SEQ, D), dtype=np.float32)
    for i in CORE_IDS:
        out[i::N_CORES] = np.asarray(results[i]["out"], dtype=np.float32).T
    return out


def kernel(x, Wq, Wk, Wv):
    global _NC_CACHE
    if _NC_CACHE is None:
        _NC_CACHE = build_nc()
    in_maps = make_in_maps(x, Wq, Wk, Wv)
    res = run_bass_kernel_spmd(nc := _NC_CACHE, in_maps, core_ids=CORE_IDS)
    return assemble(res.results)


_NC_CACHE = None


# revision 57
# speedup vs baseline: 1.0894x; 1.0894x over previous
"""Causal attention (single head, S=4096, d=1024) on 8 TRN2 NeuronCores —
collective-free, transposed-score formulation.

Core i computes output rows {i + 8m} (strided sequence-parallel Q; perfectly
load-balanced). All cross-core communication is eliminated algebraically:
with W' := Wq^T Wk precomputed on the host (f32, cast bf16),

    S   = Q K^T = x W' x^T          (one device projection G' = x_q W')
    O   = A V   = (A x) Wv^T        (apply Wv once at the end)

Scores are computed directly TRANSPOSED (S^T[k, q] tiles, k on partitions):
    S^T tile = xt[:, di, kblk]^T-contraction with g[:, di, q-span]
so A^T is produced by exp() with no PE transposes, and feeds the AV matmul
(lhsT = A^T block) directly. Causal masking multiplies the diagonal-band
128-col region of each k-block tile by a 0/1 bf16 mask. Softmax row sums are
N=1 matmuls against a ones vector sharing the A^T weight loads of the AV
matmul; normalization scales AX rows (q on partitions) before the final
Wv projection. The final projection is split (q 0:384 early / 384:512 late)
to overlap with the last attention chunk; output is O^T in bf16.

Numerics: bf16 matmuls, f32 PSUM accumulation; exp skips max-subtraction
(scores/32 ~ N(0,1); masked entries are exactly 0 after the mask multiply).
"""

import numpy as np
import ml_dtypes

import concourse.bass as bass  # noqa: F401  (registers engines)
import concourse.mybir as mybir
from concourse import bacc, tile, masks
from concourse.bass_utils import run_bass_kernel_spmd

SEQ = 4096
D = 1024
N_CORES = 8
CORE_IDS = list(range(N_CORES))
QLOC = SEQ // N_CORES          # 512 q rows per core
NKB = SEQ // 128               # 32 k blocks of 128
OUT_SHAPE = (1024, 512)        # out dram tensor is O^T [D, QLOC] bf16
BF16 = mybir.dt.bfloat16
F32 = mybir.dt.float32
SM_SCALE = 1.0 / np.sqrt(np.float32(D))


def _emit_compute(nc, tc, pp, cp_tiles, io, rep):
    ident, maskm_sb, ones_sb, warm_sb = cp_tiles
    xq, xt, xn, wpp, wvT, maskm, out = io

    with tc.tile_pool(name="persist", bufs=1) as pers:
        g_sb = pers.tile([128, 8, QLOC], BF16, name="g_sb")      # G'^T [din, q]
        axT_sb = pers.tile([128, 8, QLOC], BF16, name="axT_sb")  # (AX)^T
        ot_sb = pers.tile([128, 8, QLOC], BF16, name="ot_sb")    # O^T

        with (
            tc.tile_pool(name="xt", bufs=1) as xtp,
            tc.tile_pool(name="xn", bufs=1) as xnp,
        ):
            xt_sb = xtp.tile([128, 8, SEQ], BF16, name="xt_sb")   # X^T d-major
            xn_sb = xnp.tile([128, NKB, D], BF16, name="xn_sb")   # X seq-major
            wv_sb = xnp.tile([128, 8, D], BF16, name="wv_sb")
            xt_v = xt.rearrange("(a p) s -> p a s", p=128)
            xn_v = xn.rearrange("(blk p) d -> p blk d", p=128)
            wv_v = wvT.rearrange("(a p) n -> p a n", p=128)

            with tc.tile_pool(name="proj", bufs=1) as wp:
                xq_sb = wp.tile([128, 8, QLOC], BF16, name="xq_sb")
                wpp_sb = wp.tile([128, 8, D], BF16, name="wpp_sb")
                # PE p-state warmup: the tensor engine clock ramps to full
                # speed only after ~3us of continuous execution. Run dummy
                # matmuls (on scratch data, results unread) while the first
                # DMAs land so real work starts at 2.4 GHz.
                for w in range(24):
                    ps_w = pp.tile([128, 512], F32, tag="o", bufs=2,
                                   name=f"ps_warm{w}_{rep}")
                    nc.tensor.matmul(ps_w[:], warm_sb[:, 0:128], warm_sb[:],
                                     start=True, stop=True)
                # critical-path DMAs first: the tiny mask, then xq + wpp
                # (halved so the first G' group starts after ~1.5MB)
                xq_v = xq.rearrange("(a p) q -> p a q", p=128)
                wpp_v = wpp.rearrange("(a p) n -> p a n", p=128)
                nc.sync.dma_start(xq_sb[:, :, 0:256], xq_v[:, :, 0:256])
                nc.scalar.dma_start(wpp_sb[:, :, 0:512], wpp_v[:, :, 0:512])
                nc.sync.dma_start(xq_sb[:, :, 256:512], xq_v[:, :, 256:512])
                nc.scalar.dma_start(wpp_sb[:, :, 512:1024],
                                    wpp_v[:, :, 512:1024])
                nc.gpsimd.dma_start(maskm_sb[:], maskm[:])

                # bulk DMAs ordered by first use, on the idle SP/Pool queues
                def xt_dma(eng, c):
                    eng.dma_start(xt_sb[:, :, 512 * c:512 * (c + 1)],
                                  xt_v[:, :, 512 * c:512 * (c + 1)])

                def xn_dma(eng, g):
                    eng.dma_start(xn_sb[:, 4 * g:4 * (g + 1), :],
                                  xn_v[:, 4 * g:4 * (g + 1), :])

                xt_dma(nc.sync, 0); xt_dma(nc.gpsimd, 1)
                xt_dma(nc.sync, 2); xt_dma(nc.gpsimd, 3)
                xn_dma(nc.sync, 0); xn_dma(nc.gpsimd, 1)
                xt_dma(nc.sync, 4); xt_dma(nc.gpsimd, 5)
                xn_dma(nc.sync, 2); xn_dma(nc.gpsimd, 3)
                xt_dma(nc.sync, 6); xt_dma(nc.gpsimd, 7)
                xn_dma(nc.sync, 4); xn_dma(nc.gpsimd, 5)
                xn_dma(nc.sync, 6); xn_dma(nc.gpsimd, 7)
                # wv is consumed only by the last-emitted final projection
                nc.sync.dma_start(wv_sb[:, :, 0:512], wv_v[:, :, 0:512])
                nc.gpsimd.dma_start(wv_sb[:, :, 512:1024],
                                    wv_v[:, :, 512:1024])

                # --- G'^T = W'^T @ x_q^T : [1024 do', 512 q]
                for gi in range(8):
                    ps = pp.tile([128, 512], F32, tag="sacc", bufs=2,
                                 name=f"ps_g{gi}_{rep}")
                    for di in range(8):
                        nc.tensor.matmul(
                            ps[:], wpp_sb[:, di, 128 * gi:128 * (gi + 1)],
                            xq_sb[:, di, :], start=(di == 0), stop=(di == 7),
                        )
                    nc.scalar.copy(g_sb[:, gi, :], ps[:])

            with tc.tile_pool(name="attn", bufs=1) as ap:
                # A^T tiles, one per 128-wide k block; tile kb covers q-span
                # [128*(kb//8), 512): chunks kb//8..3 all need this k block.
                at = [
                    ap.tile([128, 512 - 128 * (kb // 8)], BF16,
                            name=f"at{kb}_{rep}")
                    for kb in range(NKB)
                ]
                rinv_all = ap.tile([128, 4], F32, name=f"rinv_all_{rep}")

                # band block j: columns qf < 16j of the band region are fully
                # masked; skip them in the matmul and zero them once so the
                # AV/sum reads see exact zeros.
                for kb in range(NKB):
                    j = kb % 8
                    if j > 0:
                        nc.vector.memset(at[kb][:, 0:16 * j], 0.0)

                def emit_scores(kb):
                    qlo = 128 * (kb // 8)
                    j = kb % 8
                    sk = 16 * j          # fully-masked leading band columns
                    nq = 512 - qlo - sk
                    ps_s = pp.tile([128, 512], F32, tag="sacc", bufs=2,
                                   name=f"ps_s{kb}_{rep}")
                    for di in range(8):
                        nc.tensor.matmul(
                            ps_s[:, 0:nq],
                            xt_sb[:, di, 128 * kb:128 * (kb + 1)],
                            g_sb[:, di, qlo + sk:512],
                            start=(di == 0), stop=(di == 7),
                        )
                    nc.scalar.activation(
                        at[kb][:, sk:512 - qlo], ps_s[:, 0:nq],
                        mybir.ActivationFunctionType.Exp, scale=float(SM_SCALE),
                    )
                    # partially-masked remainder of the diagonal band
                    nc.vector.tensor_mul(
                        at[kb][:, sk:128], at[kb][:, sk:128],
                        maskm_sb[:, j, sk:128],
                    )

                def emit_ax(b):
                    nkc = 8 * (b + 1)
                    ps_h0 = pp.tile([128, 512], F32, tag="ax0", bufs=1,
                                    name=f"ps_h0_{b}_{rep}")
                    ps_h1 = pp.tile([128, 512], F32, tag="ax1", bufs=1,
                                    name=f"ps_h1_{b}_{rep}")
                    ps_sum = pp.tile([128, 1], F32, tag="sum", bufs=1,
                                     name=f"ps_sum{b}_{rep}")
                    for kc in range(nkc):
                        qoff = 128 * b - 128 * (kc // 8)
                        lhs = at[kc][:, qoff:qoff + 128]
                        st, sp = (kc == 0), (kc == nkc - 1)
                        # sum first: it shares the loaded A^T weights with
                        # h0/h1, and the reciprocal overlaps the last two
                        nc.tensor.matmul(ps_sum[:], lhs, ones_sb[:],
                                         start=st, stop=sp)
                        nc.tensor.matmul(ps_h0[:], lhs, xn_sb[:, kc, 0:512],
                                         start=st, stop=sp)
                        nc.tensor.matmul(ps_h1[:], lhs, xn_sb[:, kc, 512:1024],
                                         start=st, stop=sp)
                    nc.vector.reciprocal(rinv_all[:, b:b + 1], ps_sum[:])
                    ax = ap.tile([128, D], BF16, tag="ax", bufs=2,
                                 name=f"ax{b}_{rep}")
                    nc.vector.tensor_scalar_mul(ax[:, 0:512], ps_h0[:],
                                                rinv_all[:, b:b + 1])
                    nc.vector.tensor_scalar_mul(ax[:, 512:1024], ps_h1[:],
                                                rinv_all[:, b:b + 1])
                    # transpose ax -> (AX)^T column block b (PE + identity)
                    for g2 in range(2):
                        ps_t = pp.tile([128, 512], BF16, tag="t", bufs=1,
                                       name=f"ps_t{b}_{g2}_{rep}")
                        for j in range(4):
                            nc.tensor.transpose(
                                ps_t[:, 128 * j:128 * (j + 1)],
                                ax[:, 512 * g2 + 128 * j:
                                   512 * g2 + 128 * (j + 1)],
                                ident[:],
                            )
                        nc.vector.tensor_copy(
                            axT_sb[:, 4 * g2:4 * (g2 + 1),
                                   128 * b:128 * (b + 1)],
                            ps_t[:].rearrange("p (j c) -> p j c", j=4),
                        )
                    return ax

                def emit_final(q0, q1, do):
                    # alternate PSUM tags: 4 rotating banks ("o" + the
                    # scores-stream banks, free by now) so short N=128
                    # groups aren't gated on the PSUM->SBUF copies
                    tag = "o" if do % 2 == 0 else "sacc"
                    ps = pp.tile([128, 512], F32, tag=tag, bufs=2,
                                 name=f"ps_ot{do}_{q0}_{rep}")
                    for di in range(8):
                        nc.tensor.matmul(
                            ps[:, 0:q1 - q0],
                            wv_sb[:, di, 128 * do:128 * (do + 1)],
                            axT_sb[:, di, q0:q1],
                            start=(di == 0), stop=(di == 7),
                        )
                    if do % 2 == 0:
                        nc.scalar.copy(ot_sb[:, do, q0:q1], ps[:, 0:q1 - q0])
                    else:
                        nc.vector.tensor_copy(ot_sb[:, do, q0:q1],
                                              ps[:, 0:q1 - q0])

                # pipeline: scores stream (sums lagged one block) with
                # per-chunk AV / transposes interleaved
                for kb in range(8):
                    emit_scores(kb)
                emit_ax(0)
                for kb in range(8, 16):
                    emit_scores(kb)
                emit_ax(1)
                for kb in range(16, 24):
                    emit_scores(kb)
                emit_ax(2)
                for kb in range(24, 32):
                    emit_scores(kb)
                # final projection for q 0:384 overlaps the last chunk;
                # its output DMAs drain during AX(3)
                out_v = out.rearrange("(a p) q -> p a q", p=128)
                for do in range(8):
                    emit_final(0, 384, do)
                    eng = nc.sync if do % 2 == 0 else nc.gpsimd
                    eng.dma_start(out_v[:, do, 0:384], ot_sb[:, do, 0:384])
                emit_ax(3)
                for do in range(8):
                    emit_final(384, 512, do)
                    eng = nc.sync if do % 2 == 0 else nc.gpsimd
                    eng.dma_start(out_v[:, do, 384:512], ot_sb[:, do, 384:512])
                if _DBG_SINK[0] is not None:
                    nc.sync.dma_start(_DBG_SINK[0][:], rinv_all[:])


_DBG_SINK = [None]


def build_nc(reps=1, variant="full", loop=False, debug=False):
    nc = bacc.Bacc("TRN2", target_bir_lowering=False)
    _DBG_SINK[0] = (
        nc.dram_tensor("dbg", [128, 4], F32, kind="ExternalOutput")
        if debug else None
    )

    xq = nc.dram_tensor("xq", [D, QLOC], BF16, kind="ExternalInput")
    xt = nc.dram_tensor("xt", [D, SEQ], BF16, kind="ExternalInput")
    xn = nc.dram_tensor("xn", [SEQ, D], BF16, kind="ExternalInput")
    wpp = nc.dram_tensor("wpp", [D, D], BF16, kind="ExternalInput")
    wvT = nc.dram_tensor("wvT", [D, D], BF16, kind="ExternalInput")
    maskm = nc.dram_tensor("maskm", [128, 1024], BF16, kind="ExternalInput")
    out = nc.dram_tensor("out", [D, QLOC], BF16, kind="ExternalOutput")
    io = (xq, xt, xn, wpp, wvT, maskm, out)

    with tile.TileContext(nc) as tc:
        with (
            tc.tile_pool(name="const", bufs=1) as cp,
            tc.tile_pool(name="psum", bufs=2, space="PSUM") as pp,
        ):
            ident = cp.tile([128, 128], BF16, name="ident")
            masks.make_identity(nc, ident[:])
            ones_sb = cp.tile([128, 1], BF16, name="ones_sb")
            nc.vector.memset(ones_sb[:], 1.0)
            maskm_sb = cp.tile([128, 8, 128], BF16, name="maskm_sb")
            # scratch operand for the p-state warmup matmuls (results unread)
            warm_sb = cp.tile([128, 512], BF16, name="warm_sb")
            nc.vector.memset(warm_sb[:], 0.0)
            cp_tiles = (ident, maskm_sb, ones_sb, warm_sb)
            if loop:
                # hardware loop: body repeats `reps` times, all-engine
                # barrier between iterations (in For_i's reset block)
                with tc.For_i(0, reps):
                    _emit_compute(nc, tc, pp, cp_tiles, io, 0)
            else:
                for rep in range(reps):
                    if rep > 0:
                        # serialize reps so the R-slope measures single-shot
                        tc.strict_bb_all_engine_barrier()
                    _emit_compute(nc, tc, pp, cp_tiles, io, rep)

    nc.compile()
    return nc


def make_in_maps(x, Wq, Wk, Wv):
    x = np.asarray(x, dtype=np.float32)
    Wq = np.asarray(Wq, dtype=np.float32)
    Wk = np.asarray(Wk, dtype=np.float32)
    Wv = np.asarray(Wv, dtype=np.float32)

    bf = ml_dtypes.bfloat16
    xT = np.ascontiguousarray(x.T).astype(bf)          # [D, SEQ]
    xn = np.ascontiguousarray(x).astype(bf)            # [SEQ, D]
    wpp = np.ascontiguousarray(Wq.T @ Wk).astype(bf)   # W' = Wq^T Wk [din,din]
    wvT = np.ascontiguousarray(Wv.T).astype(bf)

    # 0/1 band mask, S^T orientation: for band block j (k rows 128j+p of the
    # chunk's 1024-span), q col qf (global q = 8*qf + i within the span):
    # valid iff 128j + p <= 8*qf + i.
    p = np.arange(128)[:, None, None]
    j = np.arange(8)[None, :, None]
    qf = np.arange(128)[None, None, :]
    in_maps = []
    for i in CORE_IDS:
        m = (128 * j + p <= 8 * qf + i).astype(bf).reshape(128, 1024)
        in_maps.append({
            "xq": np.ascontiguousarray(xT[:, i::N_CORES]),
            "xt": xT, "xn": xn,
            "wpp": wpp, "wvT": wvT,
            "maskm": np.ascontiguousarray(m),
        })
    return in_maps


def assemble(results):
    out = np.empty((SEQ, D), dtype=np.float32)
    for i in CORE_IDS:
        out[i::N_CORES] = np.asarray(results[i]["out"], dtype=np.float32).T
    return out


def kernel(x, Wq, Wk, Wv):
    global _NC_CACHE
    if _NC_CACHE is None:
        _NC_CACHE = build_nc()
    in_maps = make_in_maps(x, Wq, Wk, Wv)
    res = run_bass_kernel_spmd(nc := _NC_CACHE, in_maps, core_ids=CORE_IDS)
    return assemble(res.results)


_NC_CACHE = None
